# revision 1
# baseline (speedup 1.0000x reference)
"""DNGPU cell (gated conv recurrence) for Trainium2, data-parallel over batch on 8 cores.

Problem: B=32, L=128, C=192, K=3; 128 sequential steps of
    reset = sigmoid(conv(mem, w_reset) + 0.5)
    gate  = sigmoid(conv(mem, w_gate) + 0.7)
    cand  = tanh(conv(reset*mem, w_cand))
    mem   = gate*shift_right(mem) + (1-gate)*cand

Per-core layout: state held in SBUF as [C partitions, token cols] where
token col = 4 + l*4 + b  (l-major, b-minor, 4 zero-pad cols each side).
Conv taps are column-shifted views (tap k reads cols k*4 .. k*4+512), and
shift_right is the view shifted by -4. C=192 splits into an A half
(channels 0:128) and B half (128:192). Matmuls run in float32r (full PE
rate at N>=256, ~1.5e-4 per-matmul rel err measured on HW).
"""

import numpy as np
from contextlib import ExitStack

import concourse.bacc as bacc
import concourse.tile as tile
from concourse import mybir
from concourse.tile import add_dep_helper
from concourse.bass_utils import run_bass_kernel_spmd

B, L, C = 32, 128, 192
NCORES = 8
BLOC = B // NCORES          # 4 batches per core
TOK = BLOC * L              # 512 tokens per core
WPAD = TOK + 8              # 4 zero cols each side
STEPS = 128

F32 = mybir.dt.float32
F32R = mybir.dt.float32r
AF = mybir.ActivationFunctionType

# (start, len) for the channel halves
CH = [(0, 128), (128, 64)]


def build(steps=STEPS):
    nc = bacc.Bacc("TRN2", target_bir_lowering=False, debug=False,
                   num_devices=NCORES)
    x_d = nc.dram_tensor("x", [BLOC, L, C], F32, kind="ExternalInput").ap()
    w_d = {}
    b_d = {}
    for cv, wn, bn in (("r", "w_reset", "b_reset"),
                       ("g", "w_gate", "b_gate"),
                       ("n", "w_cand", "b_cand")):
        w_d[cv] = nc.dram_tensor(wn, [3, C, C], F32, kind="ExternalInput").ap()
        b_d[cv] = nc.dram_tensor(bn, [C], F32, kind="ExternalInput").ap()
    id_d = nc.dram_tensor("ident", [128, 128], F32, kind="ExternalInput").ap()
    out_d = nc.dram_tensor("out", [BLOC, L, C], F32, kind="ExternalOutput").ap()

    with tile.TileContext(nc) as tc, ExitStack() as ctx:
        const = ctx.enter_context(tc.tile_pool(name="const", bufs=1))
        state = ctx.enter_context(tc.tile_pool(name="state", bufs=1))
        act = ctx.enter_context(tc.tile_pool(name="act", bufs=6))
        tmp = ctx.enter_context(tc.tile_pool(name="tmp", bufs=4))
        psum = ctx.enter_context(tc.tile_pool(name="psum", bufs=1, space="PSUM"))

        # --- constants ---------------------------------------------------
        w = {}
        for cv in ("r", "g", "n"):
            for k in range(3):
                for ci, (c0, cl) in enumerate(CH):
                    for co, (o0, ol) in enumerate(CH):
                        t = const.tile([cl, ol], F32R, tag=f"w{cv}{k}{ci}{co}")
                        nc.gpsimd.dma_start(t[:], w_d[cv][k, c0:c0 + cl, o0:o0 + ol])
                        w[cv, k, ci, co] = t
        wp = {}
        for cv in ("r", "g", "n"):
            for co, (o0, ol) in enumerate(CH):
                t = const.tile([128, ol], F32R, tag=f"wp{cv}{co}", name=f"wp{cv}{co}")
                nc.gpsimd.dma_start(t[0:64, :], w_d[cv][0, 128:192, o0:o0 + ol])
                nc.gpsimd.dma_start(t[64:128, :], w_d[cv][1, 128:192, o0:o0 + ol])
                wp[cv, co] = t
        whi = {}
        for cv, co in (("g", 0), ("g", 1), ("n", 1)):
            o0, ol = CH[co]
            t = const.tile([128, ol], F32R, tag=f"whi{cv}{co}", name=f"whi{cv}{co}")
            nc.gpsimd.dma_start(t[64:128, :], w_d[cv][2, 128:192, o0:o0 + ol])
            whi[cv, co] = t
        bias = {}
        for cv in ("r", "g"):
            for ci, (c0, cl) in enumerate(CH):
                t = const.tile([cl, 1], F32, tag=f"b{cv}{ci}")
                nc.sync.dma_start(t[:, 0], b_d[cv][c0:c0 + cl])
                bias[cv, ci] = t
        ident = const.tile([128, 128], F32, tag="ident")
        nc.sync.dma_start(ident[:], id_d)
        identr = const.tile([128, 128], F32R, tag="identr")
        nc.gpsimd.dma_start(identr[:], id_d)

        # --- state tiles ---------------------------------------------------
        mem = {}
        for i in range(2):
            mem[i, 0] = state.tile([128, WPAD], F32R, tag=f"memA{i}", name=f"memA{i}")
            mem[i, 1] = state.tile([128, WPAD], F32R, tag=f"memB{i}", name=f"memB{i}")
        rmem = {0: state.tile([128, WPAD], F32R, tag="rmemA", name="rmemA"),
                1: state.tile([128, WPAD], F32R, tag="rmemB", name="rmemB")}
        zf32 = state.tile([128, WPAD], F32, tag="zf32", name="zf32")
        nc.gpsimd.memset(zf32[:], 0.0)
        for t in list(mem.values()) + list(rmem.values()):
            p = t.shape[0]
            nc.vector.tensor_copy(t[:], zf32[0:p, :])

        # --- input transform: x[b,l,c] -> mem[0] = [c, 4+l*4+b] ------------
        for b in range(BLOC):
            xb = tmp.tile([L, C], F32, tag="xload")
            nc.sync.dma_start(xb[:], x_d[b])
            for ci, (c0, cl) in enumerate(CH):
                ps = psum.tile([cl, L], F32, tag=f"tp{ci}")
                nc.tensor.transpose(ps[:], xb[:, c0:c0 + cl], ident[:])
                dst = mem[0, ci][0:cl, 4 + b: 4 + b + 4 * L: 4]
                nc.vector.tensor_copy(dst, ps[:])

        # initial shifted duplicate for the packed-tap cinB contraction
        nc.vector.tensor_copy(mem[0, 1][64:128, 0:TOK],
                              mem[0, 1][0:64, 4:4 + TOK])

        # --- recurrence -----------------------------------------------------
        cur = 0
        for t in range(steps):
            mcur = (mem[cur, 0], mem[cur, 1])
            mnxt = (mem[1 - cur, 0], mem[1 - cur, 1])

            # reset & gate convs: psum[cout_half, 512].
            # Emit all cinA (memA-reading) matmuls before any cinB ones:
            # memB of this step is produced by the previous step's B-half
            # combine (on GpSimd, slower), so the PE can restart on the A
            # half while that finishes.
            ps = {}
            for cv in ("r", "g"):
                for co, (o0, ol) in enumerate(CH):
                    p = psum.tile([ol, TOK], F32, tag=f"{cv}_M{co}")
                    ps[cv, co] = p
            def rg_cina(cv, co):
                for k in range(3):
                    nc.tensor.matmul(
                        ps[cv, co][:], w[cv, k, 0, co][:],
                        mcur[0][:, k * 4: k * 4 + TOK],
                        start=(k == 0), stop=False)

            def rg_cinb(cv, co):
                nc.tensor.matmul(ps[cv, co][:], wp[cv, co][:],
                                 mcur[1][:, 0:TOK], start=False, stop=False)
                nc.tensor.matmul(ps[cv, co][:], w[cv, 2, 1, co][:],
                                 mcur[1][0:64, 8:8 + TOK],
                                 start=False, stop=True)

            rg_cina("r", 0)
            rg_cina("r", 1)
            rg_cina("g", 0)
            rg_cinb("r", 0)
            rg_cina("g", 1)
            rg_cinb("r", 1)
            rg_cinb("g", 0)
            rg_cinb("g", 1)

            sig = {}
            for cv in ("r", "g"):
                for co, (o0, ol) in enumerate(CH):
                    s = act.tile([ol, TOK], F32R, tag=f"s{cv}{co}")
                    nc.scalar.activation(s[:], ps[cv, co][:], AF.Sigmoid,
                                         bias=bias[cv, co][:, 0:1])
                    sig[cv, co] = s

            # rmem = sigmoid(reset) * mem
            nc.vector.tensor_mul(rmem[0][:, 4:4 + TOK], sig["r", 0][:],
                                 mcur[0][:, 4:4 + TOK])
            nc.vector.tensor_mul(rmem[1][0:64, 4:4 + TOK], sig["r", 1][:],
                                 mcur[1][0:64, 4:4 + TOK])
            nc.vector.tensor_copy(rmem[1][64:128, 0:TOK], rmem[1][0:64, 4:4 + TOK])

            # u = gate * shifted runs early: no dependency on the cand conv
            u = {}
            for ci, (c0, cl) in enumerate(CH):
                ut = tmp.tile([cl, TOK], F32R, tag=f"u{ci}", name=f"u{ci}")
                nc.vector.tensor_mul(ut[:], sig["g", ci][:],
                                     mcur[ci][0:cl, 0:TOK])
                u[ci] = ut

            # cand conv (cinA matmuls first, as above)
            cand = {}
            psn = {}
            for co, (o0, ol) in enumerate(CH):
                psn[co] = psum.tile([ol, TOK], F32, tag=f"n_M{co}", name=f"psn{co}")
            for co in range(2):
                for k in range(3):
                    nc.tensor.matmul(
                        psn[co][:], w["n", k, 0, co][:],
                        rmem[0][:, k * 4: k * 4 + TOK],
                        start=(k == 0), stop=False)
                nc.tensor.matmul(psn[co][:], wp["n", co][:],
                                 rmem[1][:, 0:TOK], start=False, stop=False)
                nc.tensor.matmul(psn[co][:], w["n", 2, 1, co][:],
                                 rmem[1][0:64, 8:8 + TOK],
                                 start=False, stop=True)
            for co, (o0, ol) in enumerate(CH):
                c = act.tile([ol, TOK], F32R, tag=f"cd{co}")
                nc.scalar.activation(c[:], psn[co][:], AF.Tanh)
                cand[co] = c

            # keep-warm: dummy matmuls fill the PE idle tail while Vector
            # finishes the combine, so HAM doesn't re-throttle each step
            dummy = psum.tile([128, 384], F32, tag="tp1", name=f"dm{t}")
            for dk in range(5):
                nc.tensor.matmul(dummy[:], w["n", 0, 0, 0][:],
                                 mcur[0][:, 0:384], start=True, stop=True)

            # mem_next = u - (gate-1)*cand  (= gate*shifted + (1-gate)*cand)
            # Post-tanh dependency depth is 2 Vector ops per half.
            prev_sub = None
            for ci, (c0, cl) in enumerate(CH):
                q = tmp.tile([cl, TOK], F32R, tag=f"q{ci}", name=f"q{ci}")
                qi = nc.vector.scalar_tensor_tensor(
                    q[:], sig["g", ci][:], 1.0, cand[ci][:],
                    op0=mybir.AluOpType.subtract, op1=mybir.AluOpType.mult)
                if prev_sub is not None:
                    add_dep_helper(qi.ins, prev_sub.ins, sync=False,
                                   reason="finish A half before B half")
                prev_sub = nc.vector.tensor_sub(mnxt[ci][0:cl, 4:4 + TOK],
                                                u[ci][:], q[:])
                if ci == 1:
                    nc.vector.tensor_copy(mnxt[1][64:128, 0:TOK],
                                          mnxt[1][0:64, 4:4 + TOK])

            cur = 1 - cur

        # --- output transform: mem[cur] -> out[b,l,c] -----------------------
        for b in range(BLOC):
            osb = tmp.tile([L, C], F32, tag="oload")
            for ci, (c0, cl) in enumerate(CH):
                ps = psum.tile([L, cl], F32R, tag=f"tp{ci}")
                nc.tensor.transpose(ps[:], mem[cur, ci][0:cl, 4 + b: 4 + b + 4 * L: 4],
                                    identr[0:cl, 0:cl])
                nc.vector.tensor_copy(osb[:, c0:c0 + cl], ps[:])
            nc.sync.dma_start(out_d[b], osb[:])

    nc.compile()
    return nc


_built = {}


def _get(steps=STEPS):
    if steps not in _built:
        _built[steps] = build(steps)
    return _built[steps]


def kernel(x, w_reset, b_reset, w_gate, b_gate, w_cand, b_cand, steps=STEPS,
           trace=False):
    nc = _get(steps)
    ident = np.eye(128, dtype=np.float32)
    base = {"w_reset": np.asarray(w_reset, np.float32),
            "b_reset": np.asarray(b_reset, np.float32),
            "w_gate": np.asarray(w_gate, np.float32),
            "b_gate": np.asarray(b_gate, np.float32),
            "w_cand": np.asarray(w_cand, np.float32),
            "b_cand": np.asarray(b_cand, np.float32),
            "ident": ident}
    x = np.asarray(x, np.float32)
    in_maps = [dict(base, x=np.ascontiguousarray(x[i * BLOC:(i + 1) * BLOC]))
               for i in range(NCORES)]
    res = run_bass_kernel_spmd(nc, in_maps, core_ids=list(range(NCORES)),
                               trace=trace)
    out = np.concatenate([res.results[i]["out"] for i in range(NCORES)], axis=0)
    if trace:
        return out, res
    return out


if __name__ == "__main__":
    rng = np.random.default_rng(0)
    scale = 1.0 / np.sqrt(3 * C)
    ins = {
        "x": rng.standard_normal((B, L, C), dtype=np.float32),
        "w_reset": (rng.standard_normal((3, C, C)) * scale).astype(np.float32),
        "b_reset": np.full(C, 0.5, np.float32),
        "w_gate": (rng.standard_normal((3, C, C)) * scale).astype(np.float32),
        "b_gate": np.full(C, 0.7, np.float32),
        "w_cand": (rng.standard_normal((3, C, C)) * scale).astype(np.float32),
        "b_cand": np.zeros(C, np.float32),
    }
    out = kernel(**ins, steps=2)
    print("smoke ok", out.shape, out.dtype)

